# revision 20
# baseline (speedup 1.0000x reference)
"""FP64->FP32 bit-circuit converter for Trainium2 (8 NeuronCores), packed I/O.

The end-to-end cost of kernel() is dominated by host<->device transport over
the axon tunnel (~14ms/MB up, ~22ms/MB down, ~85ms RTT per synchronization;
all transfer/exec dispatch is async, so a pack->upload->exec->download chain
pays one RTT).  Device execution of the whole conversion is ~47us.
Throughput is limited PER CONNECTION, so a worker subprocess with its own
axon connection doubles aggregate wire bandwidth.  Strategy (pure data
parallel over the batch):

  host:   pack each row's 64 {0,1}-float bits into 5 bytes: the first fp64
          word (sign+exp11+mant0..19) as one int32, plus one byte holding
          mant20..23 and the sticky bit (OR of mant24..51, reduced on host
          so 28 bits collapse to 1) -> 5MB up instead of 8MB;
  device: run the full conversion circuit (RNE rounding, exponent rebias +
          carry, overflow/underflow/NaN/Inf muxes) as ~34 int32 ALU ops per
          row on the vector engine, emitting the IEEE fp32 bit pattern as
          one int32 per row (4MB back);
  host:   expand words into the (B, 32) float bit matrix via unpackbits and
          a fused multiply-by-0x3F800000 directly into the output buffer.

The batch is split into N_CHUNK chunks, alternately owned by this process
and the worker subprocess (chunks handed over via shared memory).  All CPU
work (XLA pack, numpy unpack) stays on the main process -- the VM has one
vCPU -- while the two processes' wires stream concurrently.

The Bass kernel is compiled and first executed via
bass_utils.run_bass_kernel_spmd (during warm-up, which also cross-checks the
jit fast path against it); steady-state calls reuse cached executors.
Warm-up starts in a background thread at import.  jemalloc page decay is
disabled so the 128MB output buffer reuses warm pages across calls
(~50ms/call of page faults otherwise on this 1-vCPU VM).
"""
import ctypes
import os
import queue
import subprocess
import sys
import threading
from concurrent.futures import ThreadPoolExecutor
from multiprocessing import shared_memory
from multiprocessing.connection import Client, Listener
import numpy as np

_IS_WORKER = os.environ.get("AXON_KERNEL_WORKER") == "1"


def _disable_jemalloc_decay():
    try:
        lib = ctypes.CDLL(None)
        mallctl = lib.mallctl
        mallctl.argtypes = [ctypes.c_char_p, ctypes.c_void_p,
                            ctypes.POINTER(ctypes.c_size_t),
                            ctypes.c_void_p, ctypes.c_size_t]
        mallctl.restype = ctypes.c_int

        def set_ssize(name, value):
            v = ctypes.c_ssize_t(value)
            return mallctl(name.encode(), None, None,
                           ctypes.byref(v), ctypes.sizeof(v))

        n = ctypes.c_uint(0)
        sz = ctypes.c_size_t(ctypes.sizeof(n))
        if mallctl(b"arenas.narenas", ctypes.byref(n), ctypes.byref(sz),
                   None, 0) == 0:
            for i in range(n.value):
                set_ssize(f"arena.{i}.dirty_decay_ms", -1)
                set_ssize(f"arena.{i}.muzzy_decay_ms", -1)
        set_ssize("arenas.dirty_decay_ms", -1)
        set_ssize("arenas.muzzy_decay_ms", -1)
    except Exception:
        pass


_disable_jemalloc_decay()

import jax                                              # noqa: E402
import jax.numpy as jnp                                 # noqa: E402
from jax.sharding import Mesh, PartitionSpec, NamedSharding  # noqa: E402
from jax.experimental.shard_map import shard_map        # noqa: E402

from concourse import bacc, bass2jax, mybir             # noqa: E402
from concourse.tile import TileContext                  # noqa: E402
from concourse.bass_utils import run_bass_kernel_spmd   # noqa: E402

AOT = mybir.AluOpType
I32 = mybir.dt.int32
U8 = mybir.dt.uint8

B = 1_048_576
N_CORES = 8
N_CHUNK = int(os.environ.get("BASS_NCHUNK", "8"))
RC = B // N_CHUNK              # rows per chunk (global)
RCC = RC // N_CORES            # rows per core per chunk
P = 128                        # SBUF partitions
NI = RCC // P                  # columns per partition
USE_WORKER = os.environ.get("BASS_NOWORKER", "0") != "1"
_KERNEL_PATH = os.path.abspath(__file__)


def _build():
    nc = bacc.Bacc("TRN2")
    xh = nc.dram_tensor("xh", [RCC, 1], I32, kind="ExternalInput")
    xb = nc.dram_tensor("xb", [RCC, 1], U8, kind="ExternalInput")
    y = nc.dram_tensor("y", [RCC, 1], I32, kind="ExternalOutput")
    hr = xh.ap().rearrange("(p n) d -> p (n d)", p=P)
    br = xb.ap().rearrange("(p n) d -> p (n d)", p=P)
    yr = y.ap().rearrange("(p n) d -> p (n d)", p=P)

    with TileContext(nc) as tc:
        with (
            tc.tile_pool(name="io", bufs=1) as io,
            tc.tile_pool(name="sc", bufs=1) as sc,
        ):
            ht = io.tile([P, NI], I32, tag="ht", name="ht")
            bt8 = io.tile([P, NI], U8, tag="bt8", name="bt8")
            nc.sync.dma_start(ht[:, :], hr[:, :])
            nc.sync.dma_start(bt8[:, :], br[:, :])
            hi = ht[:, :]

            def T(tag):
                t = sc.tile([P, NI], I32, tag=tag, name=tag)
                return t[:, :]

            V = nc.vector
            # widen the 5-bit byte (m20 m21 m22 m23 sticky) to int32
            bi = T("bi")
            V.tensor_scalar(bi, bt8[:, :], 0, None, AOT.add)
            # field extraction
            E = T("E")            # 11-bit biased fp64 exponent
            V.tensor_scalar(E, hi, 20, 0x7FF,
                            AOT.logical_shift_right, AOT.bitwise_and)
            Mh = T("Mh")          # mant0..19 pre-shifted into bits 3..22
            V.tensor_scalar(Mh, hi, 0xFFFFF, 3,
                            AOT.bitwise_and, AOT.logical_shift_left)
            b2 = T("b2")          # mant20..22
            V.tensor_scalar(b2, bi, 2, None, AOT.logical_shift_right)
            M23 = T("M23")        # top 23 mantissa bits as an int
            V.tensor_tensor(M23, Mh, b2, AOT.bitwise_or)
            R = T("R")            # round bit (mant23)
            V.tensor_scalar(R, bi, 1, 1,
                            AOT.logical_shift_right, AOT.bitwise_and)
            # round-to-nearest-even: ru = R & (S | L), L = lsb(M23), S = lsb(bi)
            t0 = T("t0")
            V.tensor_tensor(t0, bi, M23, AOT.bitwise_or)
            SL = T("SL")
            V.tensor_scalar(SL, t0, 1, None, AOT.bitwise_and)
            ru = T("ru")
            V.tensor_tensor(ru, R, SL, AOT.bitwise_and)
            Mr = T("Mr")
            V.tensor_tensor(Mr, M23, ru, AOT.add)
            cm = T("cm")          # mantissa carry into the exponent
            V.tensor_scalar(cm, Mr, 23, None, AOT.logical_shift_right)
            mf = T("mf")
            V.tensor_scalar(mf, Mr, 0x7FFFFF, None, AOT.bitwise_and)
            # rebias: newE = (E - 896) + cm
            nE = T("nE")
            V.scalar_tensor_tensor(nE, E, -896, cm, AOT.add, AOT.add)
            ns = T("ns")
            V.tensor_scalar(ns, nE, 23, None, AOT.logical_shift_left)
            body = T("body")
            V.tensor_tensor(body, ns, mf, AOT.bitwise_or)
            # specials
            ov = T("ov")
            V.tensor_scalar(ov, E, 1151, None, AOT.is_ge)
            un = T("un")
            V.tensor_scalar(un, E, 897, None, AOT.is_lt)
            anyv = T("anyv")      # any of the 52 mantissa bits
            V.tensor_tensor(anyv, Mh, bi, AOT.bitwise_or)
            mny = T("mny")
            V.tensor_scalar(mny, anyv, 1, None, AOT.min)
            eq = T("eq")
            V.tensor_scalar(eq, E, 2047, None, AOT.is_equal)
            nan = T("nan")
            V.tensor_tensor(nan, eq, mny, AOT.bitwise_and)
            om = T("om")
            V.tensor_scalar(om, ov, 1, None, AOT.subtract)
            um = T("um")
            V.tensor_scalar(um, un, 1, None, AOT.subtract)
            nm = T("nm")
            V.tensor_scalar(nm, nan, 1, None, AOT.subtract)
            sb = T("sb")
            V.tensor_scalar(sb, hi, 31, 31,
                            AOT.logical_shift_right, AOT.logical_shift_left)
            # body1 = ov ? 0x7F800000 : body   (xor/and with NOT-mask)
            x1 = T("x1")
            V.tensor_scalar(x1, body, 0x7F800000, None, AOT.bitwise_xor)
            x2 = T("x2")
            V.tensor_tensor(x2, x1, om, AOT.bitwise_and)
            b1 = T("b1")
            V.tensor_scalar(b1, x2, 0x7F800000, None, AOT.bitwise_xor)
            # body2 = un ? 0 : body1
            bd2 = T("bd2")
            V.tensor_tensor(bd2, b1, um, AOT.bitwise_and)
            # body3 = nan ? 0x7FC00000 : body2
            x3 = T("x3")
            V.tensor_scalar(x3, bd2, 0x7FC00000, None, AOT.bitwise_xor)
            x4 = T("x4")
            V.tensor_tensor(x4, x3, nm, AOT.bitwise_and)
            b3 = T("b3")
            V.tensor_scalar(b3, x4, 0x7FC00000, None, AOT.bitwise_xor)
            yt = io.tile([P, NI], I32, tag="yt", name="yt")
            V.tensor_tensor(yt[:, :], b3, sb, AOT.bitwise_or)
            nc.sync.dma_start(yr[:, :], yt[:, :])
    nc.compile()
    return nc


# ---------------- host-side pack (XLA CPU) ----------------
_PACK_W = (np.uint32(1) << np.arange(31, -1, -1, dtype=np.uint32)).astype(np.int32)


def _pack_chunk_cpu(xc):
    # {0.0,1.0}f32 bitcasts to {0,0x3F800000}; bit 23 is the value
    xi = jax.lax.shift_right_logical(
        jax.lax.bitcast_convert_type(xc, jnp.int32), 23) & 1
    hi = (xi[:, :32] * _PACK_W[None, :]).sum(axis=-1, dtype=jnp.int32)
    sticky = jnp.minimum(jnp.max(xi[:, 36:64], axis=-1), 1)
    b = ((xi[:, 32] << 4) | (xi[:, 33] << 3) | (xi[:, 34] << 2)
         | (xi[:, 35] << 1) | sticky)
    return hi.reshape(-1, 1), b.astype(jnp.uint8).reshape(-1, 1)


# ---------------- host-side numpy pack / unpack (fallback + trace) ----------
_W8 = np.array([128, 64, 32, 16, 8, 4, 2, 1], dtype=np.float32)


def _pack_chunk_np(x, k):
    xc = x[k * RC:(k + 1) * RC]
    by = (xc.reshape(-1, 8) @ _W8).astype(np.uint8).reshape(-1, 8)
    hi = by[:, :4].copy().view(np.dtype(">u4")).astype("<u4").view(np.int32)
    b4 = by[:, 4]
    sticky = (((b4 & 0x0F) | by[:, 5] | by[:, 6] | by[:, 7]) != 0)
    b = ((b4 >> 4) << 1) | sticky.astype(np.uint8)
    return hi.reshape(-1, 1), b.reshape(-1, 1)


def _unpack_into(w, out_i32_rows):
    """w: (rows,1) int32 fp32 words -> writes {0,0x3F800000} into the
    (rows,32) int32 view of the output floats."""
    wbe = w.reshape(-1).view(np.uint32).astype(">u4").view(np.uint8)
    bits = np.unpackbits(wbe)
    np.multiply(bits.reshape(-1, 32), np.int32(0x3F800000),
                out=out_i32_rows, dtype=np.int32, casting="unsafe")


# ---------------- bass fast-path plumbing (shared main/worker) ----------
def _make_fast_path(nc, cross_check=True):
    """Build (and warm) the jit executor for the Bass kernel.  With
    cross_check, first run the official run_bass_kernel_spmd path and
    assert the jit path reproduces it bit-exactly.
    Returns (sharded_jit, sharding, zeros_slots)."""
    rng = np.random.default_rng(1234)
    dh = rng.integers(-2**31, 2**31, (RC, 1), dtype=np.int64).astype(np.int32)
    db = rng.integers(0, 32, (RC, 1), dtype=np.uint8)
    w_official = None
    if cross_check:
        in_maps = [{"xh": dh[c * RCC:(c + 1) * RCC],
                    "xb": db[c * RCC:(c + 1) * RCC]} for c in range(N_CORES)]
        res = run_bass_kernel_spmd(nc, in_maps, core_ids=list(range(N_CORES)))
        w_official = np.concatenate([r["y"] for r in res.results], axis=0)

    bass2jax.install_neuronx_cc_hook()
    pn = nc.partition_id_tensor.name if nc.partition_id_tensor else None
    in_names, out_names, out_avals = [], [], []
    for alloc in nc.m.functions[0].allocations:
        if not isinstance(alloc, mybir.MemoryLocationSet):
            continue
        name = alloc.memorylocations[0].name
        if alloc.kind == "ExternalInput":
            if name != pn:
                in_names.append(name)
        elif alloc.kind == "ExternalOutput":
            out_names.append(name)
            out_avals.append(jax.core.ShapedArray(
                tuple(alloc.tensor_shape), mybir.dt.np(alloc.dtype)))
    assert in_names == ["xh", "xb"], in_names
    n_params, n_outs = len(in_names), len(out_avals)
    in_names_all = in_names + out_names + ([pn] if pn else [])

    def _body(*args):
        operands = list(args)
        if pn is not None:
            operands.append(bass2jax.partition_id_tensor())
        return tuple(bass2jax._bass_exec_p.bind(
            *operands, out_avals=tuple(out_avals),
            in_names=tuple(in_names_all), out_names=tuple(out_names),
            lowering_input_output_aliases=(),
            sim_require_finite=True, sim_require_nnan=True, nc=nc))

    devices = jax.devices()[:N_CORES]
    mesh = Mesh(np.asarray(devices), ("core",))
    spec = PartitionSpec("core")
    shd = NamedSharding(mesh, spec)
    sharded = jax.jit(
        shard_map(_body, mesh=mesh, in_specs=(spec,) * (n_params + n_outs),
                  out_specs=(spec,) * n_outs, check_rep=False),
        keep_unused=True)
    g_out = (RC, *out_avals[0].shape[1:])
    zeros_jit = jax.jit(lambda: jnp.zeros(g_out, out_avals[0].dtype),
                        out_shardings=shd)
    zeros = [zeros_jit() for _ in range(N_CHUNK)]
    for z in zeros:
        z.block_until_ready()

    # warm + (optionally) cross-check the fast path against the official run
    d_h = jax.device_put(dh, shd)
    d_b = jax.device_put(db, shd)
    out = sharded(d_h, d_b, zeros[0])
    w_fast = np.asarray(out[0])
    if w_official is not None:
        assert np.array_equal(w_fast, w_official), "fast path mismatch"
    return sharded, shd, zeros


# ---------------- worker subprocess ----------------
def _worker_entry(addr, hi_name, b_name, w_name):
    conn = Client(addr)
    try:
        shm_hi = shared_memory.SharedMemory(name=hi_name, track=False)
        shm_b = shared_memory.SharedMemory(name=b_name, track=False)
        shm_w = shared_memory.SharedMemory(name=w_name, track=False)
        v_hi = np.ndarray((B, 1), np.int32, buffer=shm_hi.buf)
        v_b = np.ndarray((B, 1), np.uint8, buffer=shm_b.buf)
        v_w = np.ndarray((B, 1), np.int32, buffer=shm_w.buf)

        nc = _build()
        sharded, shd, zeros = _make_fast_path(nc, cross_check=False)
        pool = ThreadPoolExecutor(max_workers=N_CHUNK)
        send_lock = threading.Lock()

        def chain(k):
            try:
                sl = slice(k * RC, (k + 1) * RC)
                dh = jax.device_put(v_hi[sl], shd)
                db = jax.device_put(v_b[sl], shd)
                o = sharded(dh, db, zeros[k])
                v_w[sl] = np.asarray(o[0])
                with send_lock:
                    conn.send(("done", k))
            except Exception as e:
                try:
                    with send_lock:
                        conn.send(("err", repr(e)))
                except Exception:
                    pass

        conn.send(("ready",))
        while True:
            msg = conn.recv()
            if msg[0] == "chunk":
                pool.submit(chain, msg[1])
            elif msg[0] == "quit":
                break
    except (EOFError, OSError):
        pass
    except Exception as e:
        try:
            conn.send(("err", repr(e)))
        except Exception:
            pass
    os._exit(0)


# ---------------- cached executor (main process) ----------------
_STATE: dict = {}
_LOCK = threading.Lock()


def _spawn_worker():
    try:
        addr = f"/tmp/axk_{os.getpid()}.sock"
        try:
            os.unlink(addr)
        except OSError:
            pass
        listener = Listener(addr, family="AF_UNIX")
        shms = []
        for sz in (B * 4, B, B * 4):
            shms.append(shared_memory.SharedMemory(create=True, size=sz))
        env = dict(os.environ)
        env["AXON_KERNEL_WORKER"] = "1"
        env["BASS_NCHUNK"] = str(N_CHUNK)
        logf = open("/tmp/axk_worker.log", "wb")
        proc = subprocess.Popen(
            [sys.executable, _KERNEL_PATH, "--worker", addr,
             shms[0].name, shms[1].name, shms[2].name],
            env=env, stdout=logf, stderr=logf, stdin=subprocess.DEVNULL)
        _STATE["worker_proc"] = proc
        _STATE["shms"] = shms
        _STATE["w_hi"] = np.ndarray((B, 1), np.int32, buffer=shms[0].buf)
        _STATE["w_b"] = np.ndarray((B, 1), np.uint8, buffer=shms[1].buf)
        _STATE["w_w"] = np.ndarray((B, 1), np.int32, buffer=shms[2].buf)
        _STATE["wq"] = queue.Queue()

        def _accept_and_read():
            try:
                conn = listener.accept()
                _STATE["conn"] = conn
                _STATE["send_lock"] = threading.Lock()
                while True:
                    msg = conn.recv()
                    if msg[0] == "ready":
                        _STATE["worker_ok"] = True
                    elif msg[0] == "done":
                        _STATE["wq"].put(msg[1])
                    elif msg[0] == "err":
                        _STATE.pop("worker_ok", None)
                        _STATE["worker_err"] = msg[1]
                        _STATE["wq"].put(None)
            except Exception:
                _STATE.pop("worker_ok", None)
                _STATE["wq"].put(None)

        threading.Thread(target=_accept_and_read, daemon=True).start()

        import atexit

        def _cleanup():
            try:
                proc.kill()
            except Exception:
                pass
            for s in shms:
                try:
                    s.close()
                    s.unlink()
                except Exception:
                    pass

        atexit.register(_cleanup)
    except Exception as e:
        _STATE["worker_err"] = repr(e)


def _prepare_locked():
    if "ready" in _STATE or "failed" in _STATE:
        return
    try:
        if USE_WORKER and not _IS_WORKER:
            _spawn_worker()           # warms in parallel with our own warmup
        nc = _build()
        _STATE["nc"] = nc
        sharded, shd, zeros = _make_fast_path(nc)
        pack_jit = jax.jit(_pack_chunk_cpu, backend="cpu")
        pack_jit(np.zeros((RC, 64), np.float32))
        pool = ThreadPoolExecutor(max_workers=N_CHUNK)
        _STATE.update(dict(pack_jit=pack_jit, sharded=sharded,
                           zeros=zeros, shd=shd, pool=pool, ready=True))
    except Exception as e:  # fall back to the plain spmd path per call
        _STATE["failed"] = repr(e)
        if "nc" not in _STATE:
            try:
                _STATE["nc"] = _build()
            except Exception:
                pass


def _prepare():
    with _LOCK:
        _prepare_locked()


def _get_nc():
    _prepare()
    return _STATE["nc"]


if not _IS_WORKER:
    _WARM = threading.Thread(target=_prepare, daemon=True)
    _WARM.start()


def _kernel_fast(x, out, out_i):
    S = _STATE
    sharded, zeros, shd, pool = S["sharded"], S["zeros"], S["shd"], S["pool"]
    use_w = USE_WORKER and S.get("worker_ok") and "conn" in S
    q: queue.Queue = queue.Queue()
    if use_w:
        wq = S["wq"]
        while not wq.empty():        # drop stale entries from a failed call
            try:
                wq.get_nowait()
            except queue.Empty:
                break

    def chain(k, hi_np, b_np):
        dh = jax.device_put(hi_np, shd)
        db = jax.device_put(b_np, shd)
        o = sharded(dh, db, zeros[k])
        w = np.asarray(o[0])
        q.put((k, w))

    # Pack EVERYTHING first (the single vCPU packs ~4x slower once transfer
    # streams run, so interleaving pack with the wire stretches the dispatch
    # span; packing clean then dispatching all chains at once is faster).
    jobs = [S["pack_jit"](x[k * RC:(k + 1) * RC]) for k in range(N_CHUNK)]
    parts = [(np.asarray(hi), np.asarray(b)) for hi, b in jobs]
    packed = {}
    for k, (hi_np, b_np) in enumerate(parts):
        if use_w and (k % 2 == 1):
            sl = slice(k * RC, (k + 1) * RC)
            S["w_hi"][sl] = hi_np
            S["w_b"][sl] = b_np
            with S["send_lock"]:
                S["conn"].send(("chunk", k))
            packed[k] = (hi_np, b_np)
        else:
            pool.submit(chain, k, hi_np, b_np)
    # pre-fault the output pages while the wire is busy
    out.reshape(-1)[::1024] = 0.0

    n_done = 0
    worker_pending = set(packed.keys())
    while n_done < N_CHUNK:
        if worker_pending:
            # wait on whichever source delivers next
            got = None
            while got is None:
                try:
                    got = ("w", S["wq"].get(timeout=0.002))
                except queue.Empty:
                    try:
                        got = ("l", q.get_nowait())
                    except queue.Empty:
                        got = None
            src, item = got
            if src == "w":
                if item is None or not S.get("worker_ok"):
                    # worker died: re-run its remaining chunks locally
                    for k in sorted(worker_pending):
                        hi_np, b_np = packed[k]
                        pool.submit(chain, k, hi_np, b_np)
                    worker_pending.clear()
                    continue
                k = item
                worker_pending.discard(k)
                sl = slice(k * RC, (k + 1) * RC)
                _unpack_into(S["w_w"][sl], out_i[sl])
                n_done += 1
            else:
                k, w = item
                _unpack_into(w, out_i[k * RC:(k + 1) * RC])
                n_done += 1
        else:
            k, w = q.get()
            _unpack_into(w, out_i[k * RC:(k + 1) * RC])
            n_done += 1
    return out


def _wait_worker_once(timeout_s=150.0):
    """On the first call only: give the worker subprocess a bounded window
    to finish its warm-up so the steady-state calls use both connections."""
    if _STATE.get("worker_wait_done") or not USE_WORKER:
        return
    _STATE["worker_wait_done"] = True
    import time
    deadline = time.monotonic() + timeout_s
    proc = _STATE.get("worker_proc")
    while (proc is not None and not _STATE.get("worker_ok")
           and proc.poll() is None and time.monotonic() < deadline):
        time.sleep(0.1)


def kernel(fp64_pulse: np.ndarray) -> np.ndarray:
    x = np.asarray(fp64_pulse)
    assert x.shape == (B, 64)
    _prepare()
    if "ready" in _STATE:
        _wait_worker_once()
    out = np.empty((B, 32), np.float32)
    out_i = out.view(np.int32)
    if "ready" in _STATE:
        try:
            return _kernel_fast(x, out, out_i)
        except Exception:
            pass  # transient failure: serve this call via the plain path
    # fallback: plain official path with numpy pack/unpack
    nc = _STATE["nc"]
    for k in range(N_CHUNK):
        hi, b = _pack_chunk_np(x, k)
        in_maps = [{"xh": hi[c * RCC:(c + 1) * RCC],
                    "xb": b[c * RCC:(c + 1) * RCC]} for c in range(N_CORES)]
        res = run_bass_kernel_spmd(nc, in_maps, core_ids=list(range(N_CORES)))
        w = np.concatenate([r["y"] for r in res.results], axis=0)
        _unpack_into(w, out_i[k * RC:(k + 1) * RC])
    return out


if __name__ == "__main__" and len(sys.argv) >= 6 and sys.argv[1] == "--worker":
    _worker_entry(sys.argv[2], sys.argv[3], sys.argv[4], sys.argv[5])


# revision 22
# speedup vs baseline: 1.2689x; 1.2689x over previous
"""FP64->FP32 bit-circuit converter for Trainium2 (8 NeuronCores), packed I/O.

The end-to-end cost of kernel() is dominated by host<->device transport over
the axon tunnel (~14ms/MB up, ~22ms/MB down, ~85ms RTT per synchronization;
all transfer/exec dispatch is async, so a pack->upload->exec->download chain
pays one RTT).  Device execution of the whole conversion is ~47us.
Throughput is limited PER CONNECTION, so a worker subprocess with its own
axon connection doubles aggregate wire bandwidth.  Strategy (pure data
parallel over the batch):

  host:   pack each row's 64 {0,1}-float bits into 5 bytes: the first fp64
          word (sign+exp11+mant0..19) as one int32, plus one byte holding
          mant20..23 and the sticky bit (OR of mant24..51, reduced on host
          so 28 bits collapse to 1) -> 5MB up instead of 8MB;
  device: run the full conversion circuit (RNE rounding, exponent rebias +
          carry, overflow/underflow/NaN/Inf muxes) as ~34 int32 ALU ops per
          row on the vector engine, emitting the IEEE fp32 bit pattern as
          one int32 per row (4MB back);
  host:   expand words into the (B, 32) float bit matrix via unpackbits and
          a fused multiply-by-0x3F800000 directly into the output buffer.

The batch is split into N_CHUNK chunks, alternately owned by this process
and the worker subprocess (chunks handed over via shared memory).  All CPU
work (XLA pack, numpy unpack) stays on the main process -- the VM has one
vCPU -- while the two processes' wires stream concurrently.

The Bass kernel is compiled and first executed via
bass_utils.run_bass_kernel_spmd (during warm-up, which also cross-checks the
jit fast path against it); steady-state calls reuse cached executors.
Warm-up starts in a background thread at import.  jemalloc page decay is
disabled so the 128MB output buffer reuses warm pages across calls
(~50ms/call of page faults otherwise on this 1-vCPU VM).
"""
import ctypes
import os
import queue
import subprocess
import sys
import threading
from concurrent.futures import ThreadPoolExecutor
from multiprocessing import shared_memory
from multiprocessing.connection import Client, Listener
import numpy as np

_IS_WORKER = os.environ.get("AXON_KERNEL_WORKER") == "1"


def _disable_jemalloc_decay():
    try:
        lib = ctypes.CDLL(None)
        mallctl = lib.mallctl
        mallctl.argtypes = [ctypes.c_char_p, ctypes.c_void_p,
                            ctypes.POINTER(ctypes.c_size_t),
                            ctypes.c_void_p, ctypes.c_size_t]
        mallctl.restype = ctypes.c_int

        def set_ssize(name, value):
            v = ctypes.c_ssize_t(value)
            return mallctl(name.encode(), None, None,
                           ctypes.byref(v), ctypes.sizeof(v))

        n = ctypes.c_uint(0)
        sz = ctypes.c_size_t(ctypes.sizeof(n))
        if mallctl(b"arenas.narenas", ctypes.byref(n), ctypes.byref(sz),
                   None, 0) == 0:
            for i in range(n.value):
                set_ssize(f"arena.{i}.dirty_decay_ms", -1)
                set_ssize(f"arena.{i}.muzzy_decay_ms", -1)
        set_ssize("arenas.dirty_decay_ms", -1)
        set_ssize("arenas.muzzy_decay_ms", -1)
    except Exception:
        pass


_disable_jemalloc_decay()

import jax                                              # noqa: E402
import jax.numpy as jnp                                 # noqa: E402
from jax.sharding import Mesh, PartitionSpec, NamedSharding  # noqa: E402
from jax.experimental.shard_map import shard_map        # noqa: E402

from concourse import bacc, bass2jax, mybir             # noqa: E402
from concourse.tile import TileContext                  # noqa: E402
from concourse.bass_utils import run_bass_kernel_spmd   # noqa: E402

AOT = mybir.AluOpType
I32 = mybir.dt.int32
U8 = mybir.dt.uint8

B = 1_048_576
N_CORES = 8
N_CHUNK = int(os.environ.get("BASS_NCHUNK", "8"))
RC = B // N_CHUNK              # rows per chunk (global)
RCC = RC // N_CORES            # rows per core per chunk
P = 128                        # SBUF partitions
NI = RCC // P                  # columns per partition
# A second axon connection does NOT increase aggregate wire throughput (the
# download path is capped globally at ~35MB/s) and its process steals CPU,
# so the worker subprocess is disabled by default.
USE_WORKER = os.environ.get("BASS_WORKER", "0") == "1"
_KERNEL_PATH = os.path.abspath(__file__)


def _build():
    nc = bacc.Bacc("TRN2")
    xh = nc.dram_tensor("xh", [RCC, 1], I32, kind="ExternalInput")
    xb = nc.dram_tensor("xb", [RCC, 1], U8, kind="ExternalInput")
    y = nc.dram_tensor("y", [RCC, 1], I32, kind="ExternalOutput")
    hr = xh.ap().rearrange("(p n) d -> p (n d)", p=P)
    br = xb.ap().rearrange("(p n) d -> p (n d)", p=P)
    yr = y.ap().rearrange("(p n) d -> p (n d)", p=P)

    with TileContext(nc) as tc:
        with (
            tc.tile_pool(name="io", bufs=1) as io,
            tc.tile_pool(name="sc", bufs=1) as sc,
        ):
            ht = io.tile([P, NI], I32, tag="ht", name="ht")
            bt8 = io.tile([P, NI], U8, tag="bt8", name="bt8")
            nc.sync.dma_start(ht[:, :], hr[:, :])
            nc.sync.dma_start(bt8[:, :], br[:, :])
            hi = ht[:, :]

            def T(tag):
                t = sc.tile([P, NI], I32, tag=tag, name=tag)
                return t[:, :]

            V = nc.vector
            # widen the 5-bit byte (m20 m21 m22 m23 sticky) to int32
            bi = T("bi")
            V.tensor_scalar(bi, bt8[:, :], 0, None, AOT.add)
            # field extraction
            E = T("E")            # 11-bit biased fp64 exponent
            V.tensor_scalar(E, hi, 20, 0x7FF,
                            AOT.logical_shift_right, AOT.bitwise_and)
            Mh = T("Mh")          # mant0..19 pre-shifted into bits 3..22
            V.tensor_scalar(Mh, hi, 0xFFFFF, 3,
                            AOT.bitwise_and, AOT.logical_shift_left)
            b2 = T("b2")          # mant20..22
            V.tensor_scalar(b2, bi, 2, None, AOT.logical_shift_right)
            M23 = T("M23")        # top 23 mantissa bits as an int
            V.tensor_tensor(M23, Mh, b2, AOT.bitwise_or)
            R = T("R")            # round bit (mant23)
            V.tensor_scalar(R, bi, 1, 1,
                            AOT.logical_shift_right, AOT.bitwise_and)
            # round-to-nearest-even: ru = R & (S | L), L = lsb(M23), S = lsb(bi)
            t0 = T("t0")
            V.tensor_tensor(t0, bi, M23, AOT.bitwise_or)
            SL = T("SL")
            V.tensor_scalar(SL, t0, 1, None, AOT.bitwise_and)
            ru = T("ru")
            V.tensor_tensor(ru, R, SL, AOT.bitwise_and)
            Mr = T("Mr")
            V.tensor_tensor(Mr, M23, ru, AOT.add)
            cm = T("cm")          # mantissa carry into the exponent
            V.tensor_scalar(cm, Mr, 23, None, AOT.logical_shift_right)
            mf = T("mf")
            V.tensor_scalar(mf, Mr, 0x7FFFFF, None, AOT.bitwise_and)
            # rebias: newE = (E - 896) + cm
            nE = T("nE")
            V.scalar_tensor_tensor(nE, E, -896, cm, AOT.add, AOT.add)
            ns = T("ns")
            V.tensor_scalar(ns, nE, 23, None, AOT.logical_shift_left)
            body = T("body")
            V.tensor_tensor(body, ns, mf, AOT.bitwise_or)
            # specials
            ov = T("ov")
            V.tensor_scalar(ov, E, 1151, None, AOT.is_ge)
            un = T("un")
            V.tensor_scalar(un, E, 897, None, AOT.is_lt)
            anyv = T("anyv")      # any of the 52 mantissa bits
            V.tensor_tensor(anyv, Mh, bi, AOT.bitwise_or)
            mny = T("mny")
            V.tensor_scalar(mny, anyv, 1, None, AOT.min)
            eq = T("eq")
            V.tensor_scalar(eq, E, 2047, None, AOT.is_equal)
            nan = T("nan")
            V.tensor_tensor(nan, eq, mny, AOT.bitwise_and)
            om = T("om")
            V.tensor_scalar(om, ov, 1, None, AOT.subtract)
            um = T("um")
            V.tensor_scalar(um, un, 1, None, AOT.subtract)
            nm = T("nm")
            V.tensor_scalar(nm, nan, 1, None, AOT.subtract)
            sb = T("sb")
            V.tensor_scalar(sb, hi, 31, 31,
                            AOT.logical_shift_right, AOT.logical_shift_left)
            # body1 = ov ? 0x7F800000 : body   (xor/and with NOT-mask)
            x1 = T("x1")
            V.tensor_scalar(x1, body, 0x7F800000, None, AOT.bitwise_xor)
            x2 = T("x2")
            V.tensor_tensor(x2, x1, om, AOT.bitwise_and)
            b1 = T("b1")
            V.tensor_scalar(b1, x2, 0x7F800000, None, AOT.bitwise_xor)
            # body2 = un ? 0 : body1
            bd2 = T("bd2")
            V.tensor_tensor(bd2, b1, um, AOT.bitwise_and)
            # body3 = nan ? 0x7FC00000 : body2
            x3 = T("x3")
            V.tensor_scalar(x3, bd2, 0x7FC00000, None, AOT.bitwise_xor)
            x4 = T("x4")
            V.tensor_tensor(x4, x3, nm, AOT.bitwise_and)
            b3 = T("b3")
            V.tensor_scalar(b3, x4, 0x7FC00000, None, AOT.bitwise_xor)
            yt = io.tile([P, NI], I32, tag="yt", name="yt")
            V.tensor_tensor(yt[:, :], b3, sb, AOT.bitwise_or)
            nc.sync.dma_start(yr[:, :], yt[:, :])
    nc.compile()
    return nc


# ---------------- host-side pack (XLA CPU) ----------------
_PACK_W = (np.uint32(1) << np.arange(31, -1, -1, dtype=np.uint32)).astype(np.int32)


def _pack_chunk_cpu(xc):
    # {0.0,1.0}f32 bitcasts to {0,0x3F800000}; bit 23 is the value
    xi = jax.lax.shift_right_logical(
        jax.lax.bitcast_convert_type(xc, jnp.int32), 23) & 1
    hi = (xi[:, :32] * _PACK_W[None, :]).sum(axis=-1, dtype=jnp.int32)
    sticky = jnp.minimum(jnp.max(xi[:, 36:64], axis=-1), 1)
    b = ((xi[:, 32] << 4) | (xi[:, 33] << 3) | (xi[:, 34] << 2)
         | (xi[:, 35] << 1) | sticky)
    return hi.reshape(-1, 1), b.astype(jnp.uint8).reshape(-1, 1)


# ---------------- host-side numpy pack / unpack (fallback + trace) ----------
_W8 = np.array([128, 64, 32, 16, 8, 4, 2, 1], dtype=np.float32)


def _pack_chunk_np(x, k):
    xc = x[k * RC:(k + 1) * RC]
    by = (xc.reshape(-1, 8) @ _W8).astype(np.uint8).reshape(-1, 8)
    hi = by[:, :4].copy().view(np.dtype(">u4")).astype("<u4").view(np.int32)
    b4 = by[:, 4]
    sticky = (((b4 & 0x0F) | by[:, 5] | by[:, 6] | by[:, 7]) != 0)
    b = ((b4 >> 4) << 1) | sticky.astype(np.uint8)
    return hi.reshape(-1, 1), b.reshape(-1, 1)


def _unpack_into(w, out_i32_rows):
    """w: (rows,1) int32 fp32 words -> writes {0,0x3F800000} into the
    (rows,32) int32 view of the output floats."""
    wbe = w.reshape(-1).view(np.uint32).astype(">u4").view(np.uint8)
    bits = np.unpackbits(wbe)
    np.multiply(bits.reshape(-1, 32), np.int32(0x3F800000),
                out=out_i32_rows, dtype=np.int32, casting="unsafe")


# ---------------- bass fast-path plumbing (shared main/worker) ----------
def _make_fast_path(nc, cross_check=True):
    """Build (and warm) the jit executor for the Bass kernel.  With
    cross_check, first run the official run_bass_kernel_spmd path and
    assert the jit path reproduces it bit-exactly.
    Returns (sharded_jit, sharding, zeros_slots)."""
    rng = np.random.default_rng(1234)
    dh = rng.integers(-2**31, 2**31, (RC, 1), dtype=np.int64).astype(np.int32)
    db = rng.integers(0, 32, (RC, 1), dtype=np.uint8)
    w_official = None
    if cross_check:
        in_maps = [{"xh": dh[c * RCC:(c + 1) * RCC],
                    "xb": db[c * RCC:(c + 1) * RCC]} for c in range(N_CORES)]
        res = run_bass_kernel_spmd(nc, in_maps, core_ids=list(range(N_CORES)))
        w_official = np.concatenate([r["y"] for r in res.results], axis=0)

    bass2jax.install_neuronx_cc_hook()
    pn = nc.partition_id_tensor.name if nc.partition_id_tensor else None
    in_names, out_names, out_avals = [], [], []
    for alloc in nc.m.functions[0].allocations:
        if not isinstance(alloc, mybir.MemoryLocationSet):
            continue
        name = alloc.memorylocations[0].name
        if alloc.kind == "ExternalInput":
            if name != pn:
                in_names.append(name)
        elif alloc.kind == "ExternalOutput":
            out_names.append(name)
            out_avals.append(jax.core.ShapedArray(
                tuple(alloc.tensor_shape), mybir.dt.np(alloc.dtype)))
    assert in_names == ["xh", "xb"], in_names
    n_params, n_outs = len(in_names), len(out_avals)
    in_names_all = in_names + out_names + ([pn] if pn else [])

    def _body(*args):
        operands = list(args)
        if pn is not None:
            operands.append(bass2jax.partition_id_tensor())
        return tuple(bass2jax._bass_exec_p.bind(
            *operands, out_avals=tuple(out_avals),
            in_names=tuple(in_names_all), out_names=tuple(out_names),
            lowering_input_output_aliases=(),
            sim_require_finite=True, sim_require_nnan=True, nc=nc))

    devices = jax.devices()[:N_CORES]
    mesh = Mesh(np.asarray(devices), ("core",))
    spec = PartitionSpec("core")
    shd = NamedSharding(mesh, spec)
    sharded = jax.jit(
        shard_map(_body, mesh=mesh, in_specs=(spec,) * (n_params + n_outs),
                  out_specs=(spec,) * n_outs, check_rep=False),
        keep_unused=True)
    g_out = (RC, *out_avals[0].shape[1:])
    zeros_jit = jax.jit(lambda: jnp.zeros(g_out, out_avals[0].dtype),
                        out_shardings=shd)
    zeros = [zeros_jit() for _ in range(N_CHUNK)]
    for z in zeros:
        z.block_until_ready()

    # warm + (optionally) cross-check the fast path against the official run
    d_h = jax.device_put(dh, shd)
    d_b = jax.device_put(db, shd)
    out = sharded(d_h, d_b, zeros[0])
    w_fast = np.asarray(out[0])
    if w_official is not None:
        assert np.array_equal(w_fast, w_official), "fast path mismatch"
    return sharded, shd, zeros


# ---------------- worker subprocess ----------------
def _worker_entry(addr, hi_name, b_name, w_name):
    conn = Client(addr)
    try:
        shm_hi = shared_memory.SharedMemory(name=hi_name, track=False)
        shm_b = shared_memory.SharedMemory(name=b_name, track=False)
        shm_w = shared_memory.SharedMemory(name=w_name, track=False)
        v_hi = np.ndarray((B, 1), np.int32, buffer=shm_hi.buf)
        v_b = np.ndarray((B, 1), np.uint8, buffer=shm_b.buf)
        v_w = np.ndarray((B, 1), np.int32, buffer=shm_w.buf)

        nc = _build()
        sharded, shd, zeros = _make_fast_path(nc, cross_check=False)
        pool = ThreadPoolExecutor(max_workers=N_CHUNK)
        send_lock = threading.Lock()

        def chain(k):
            try:
                sl = slice(k * RC, (k + 1) * RC)
                dh = jax.device_put(v_hi[sl], shd)
                db = jax.device_put(v_b[sl], shd)
                o = sharded(dh, db, zeros[k])
                v_w[sl] = np.asarray(o[0])
                with send_lock:
                    conn.send(("done", k))
            except Exception as e:
                try:
                    with send_lock:
                        conn.send(("err", repr(e)))
                except Exception:
                    pass

        conn.send(("ready",))
        while True:
            msg = conn.recv()
            if msg[0] == "chunk":
                pool.submit(chain, msg[1])
            elif msg[0] == "quit":
                break
    except (EOFError, OSError):
        pass
    except Exception as e:
        try:
            conn.send(("err", repr(e)))
        except Exception:
            pass
    os._exit(0)


# ---------------- cached executor (main process) ----------------
_STATE: dict = {}
_LOCK = threading.Lock()


def _spawn_worker():
    try:
        addr = f"/tmp/axk_{os.getpid()}.sock"
        try:
            os.unlink(addr)
        except OSError:
            pass
        listener = Listener(addr, family="AF_UNIX")
        shms = []
        for sz in (B * 4, B, B * 4):
            shms.append(shared_memory.SharedMemory(create=True, size=sz))
        env = dict(os.environ)
        env["AXON_KERNEL_WORKER"] = "1"
        env["BASS_NCHUNK"] = str(N_CHUNK)
        logf = open("/tmp/axk_worker.log", "wb")
        proc = subprocess.Popen(
            [sys.executable, _KERNEL_PATH, "--worker", addr,
             shms[0].name, shms[1].name, shms[2].name],
            env=env, stdout=logf, stderr=logf, stdin=subprocess.DEVNULL)
        _STATE["worker_proc"] = proc
        _STATE["shms"] = shms
        _STATE["w_hi"] = np.ndarray((B, 1), np.int32, buffer=shms[0].buf)
        _STATE["w_b"] = np.ndarray((B, 1), np.uint8, buffer=shms[1].buf)
        _STATE["w_w"] = np.ndarray((B, 1), np.int32, buffer=shms[2].buf)
        _STATE["wq"] = queue.Queue()

        def _accept_and_read():
            try:
                conn = listener.accept()
                _STATE["conn"] = conn
                _STATE["send_lock"] = threading.Lock()
                while True:
                    msg = conn.recv()
                    if msg[0] == "ready":
                        _STATE["worker_ok"] = True
                    elif msg[0] == "done":
                        _STATE["wq"].put(msg[1])
                    elif msg[0] == "err":
                        _STATE.pop("worker_ok", None)
                        _STATE["worker_err"] = msg[1]
                        _STATE["wq"].put(None)
            except Exception:
                _STATE.pop("worker_ok", None)
                _STATE["wq"].put(None)

        threading.Thread(target=_accept_and_read, daemon=True).start()

        import atexit

        def _cleanup():
            try:
                proc.kill()
            except Exception:
                pass
            for s in shms:
                try:
                    s.close()
                    s.unlink()
                except Exception:
                    pass

        atexit.register(_cleanup)
    except Exception as e:
        _STATE["worker_err"] = repr(e)


def _prepare_locked():
    if "ready" in _STATE or "failed" in _STATE:
        return
    try:
        if USE_WORKER and not _IS_WORKER:
            _spawn_worker()           # warms in parallel with our own warmup
        nc = _build()
        _STATE["nc"] = nc
        sharded, shd, zeros = _make_fast_path(nc)
        pack_jit = jax.jit(_pack_chunk_cpu, backend="cpu")
        pack_jit(np.zeros((RC, 64), np.float32))
        pool = ThreadPoolExecutor(max_workers=N_CHUNK)
        _STATE.update(dict(pack_jit=pack_jit, sharded=sharded,
                           zeros=zeros, shd=shd, pool=pool, ready=True))
    except Exception as e:  # fall back to the plain spmd path per call
        _STATE["failed"] = repr(e)
        if "nc" not in _STATE:
            try:
                _STATE["nc"] = _build()
            except Exception:
                pass


def _prepare():
    with _LOCK:
        _prepare_locked()


def _get_nc():
    _prepare()
    return _STATE["nc"]


if not _IS_WORKER:
    _WARM = threading.Thread(target=_prepare, daemon=True)
    _WARM.start()


def _kernel_fast(x, out, out_i):
    S = _STATE
    sharded, zeros, shd, pool = S["sharded"], S["zeros"], S["shd"], S["pool"]
    use_w = USE_WORKER and S.get("worker_ok") and "conn" in S
    q: queue.Queue = queue.Queue()
    if use_w:
        wq = S["wq"]
        while not wq.empty():        # drop stale entries from a failed call
            try:
                wq.get_nowait()
            except queue.Empty:
                break

    def chain(k, hi_np, b_np):
        dh = jax.device_put(hi_np, shd)
        db = jax.device_put(b_np, shd)
        o = sharded(dh, db, zeros[k])
        w = np.asarray(o[0])
        q.put((k, w))

    packed = {}
    for k in range(N_CHUNK):
        hi, b = S["pack_jit"](x[k * RC:(k + 1) * RC])
        hi_np, b_np = np.asarray(hi), np.asarray(b)
        if use_w and (k % 2 == 1):
            sl = slice(k * RC, (k + 1) * RC)
            S["w_hi"][sl] = hi_np
            S["w_b"][sl] = b_np
            with S["send_lock"]:
                S["conn"].send(("chunk", k))
            packed[k] = (hi_np, b_np)
        else:
            pool.submit(chain, k, hi_np, b_np)
    # pre-fault the output pages while the wire is busy
    out.reshape(-1)[::1024] = 0.0

    n_done = 0
    worker_pending = set(packed.keys())
    while n_done < N_CHUNK:
        if worker_pending:
            # wait on whichever source delivers next
            got = None
            while got is None:
                try:
                    got = ("w", S["wq"].get(timeout=0.002))
                except queue.Empty:
                    try:
                        got = ("l", q.get_nowait())
                    except queue.Empty:
                        got = None
            src, item = got
            if src == "w":
                if item is None or not S.get("worker_ok"):
                    # worker died: re-run its remaining chunks locally
                    for k in sorted(worker_pending):
                        hi_np, b_np = packed[k]
                        pool.submit(chain, k, hi_np, b_np)
                    worker_pending.clear()
                    continue
                k = item
                worker_pending.discard(k)
                sl = slice(k * RC, (k + 1) * RC)
                _unpack_into(S["w_w"][sl], out_i[sl])
                n_done += 1
            else:
                k, w = item
                _unpack_into(w, out_i[k * RC:(k + 1) * RC])
                n_done += 1
        else:
            k, w = q.get()
            _unpack_into(w, out_i[k * RC:(k + 1) * RC])
            n_done += 1
    return out


def _wait_worker_once(timeout_s=150.0):
    """On the first call only: give the worker subprocess a bounded window
    to finish its warm-up so the steady-state calls use both connections."""
    if _STATE.get("worker_wait_done") or not USE_WORKER:
        return
    _STATE["worker_wait_done"] = True
    import time
    deadline = time.monotonic() + timeout_s
    proc = _STATE.get("worker_proc")
    while (proc is not None and not _STATE.get("worker_ok")
           and proc.poll() is None and time.monotonic() < deadline):
        time.sleep(0.1)


def kernel(fp64_pulse: np.ndarray) -> np.ndarray:
    x = np.asarray(fp64_pulse)
    assert x.shape == (B, 64)
    _prepare()
    if "ready" in _STATE:
        _wait_worker_once()
    out = np.empty((B, 32), np.float32)
    out_i = out.view(np.int32)
    if "ready" in _STATE:
        try:
            return _kernel_fast(x, out, out_i)
        except Exception:
            pass  # transient failure: serve this call via the plain path
    # fallback: plain official path with numpy pack/unpack
    nc = _STATE["nc"]
    for k in range(N_CHUNK):
        hi, b = _pack_chunk_np(x, k)
        in_maps = [{"xh": hi[c * RCC:(c + 1) * RCC],
                    "xb": b[c * RCC:(c + 1) * RCC]} for c in range(N_CORES)]
        res = run_bass_kernel_spmd(nc, in_maps, core_ids=list(range(N_CORES)))
        w = np.concatenate([r["y"] for r in res.results], axis=0)
        _unpack_into(w, out_i[k * RC:(k + 1) * RC])
    return out


if __name__ == "__main__" and len(sys.argv) >= 6 and sys.argv[1] == "--worker":
    _worker_entry(sys.argv[2], sys.argv[3], sys.argv[4], sys.argv[5])
